# revision 2
# baseline (speedup 1.0000x reference)
"""Trainium2 Bass kernel for nn_DDC2Loss: mean of strict-upper-triangle of A@A.T.

Uses the algebraic identity
    sum_{i<j} <a_i, a_j> = (||colsum(A)||^2 - sum(A*A)) / 2
so the kernel only needs a column-sum and a sum-of-squares over A.

Sharding: data-parallel over rows. Each of the 8 cores processes a
(2048, 512) shard, producing:
  - out_cs [128, 512]: per-partition partial column sums (16-deep adds)
  - out_sq [128, 8]:   per-partition/per-chunk partial sums of squares
The tiny final combine (sum over 8*128 partials, dot product) runs on host
in float64.

Per-core engine split:
  - SP/HWDGE: 8 input DMAs of 512KB (contiguous 2KB runs per partition)
  - ACT (ScalarE): Square activation with accum_out -> sum-of-squares
  - DVE (VectorE): pairwise tensor_add tree -> column-sum partial
  - PE/GPSIMD idle; no collectives, no cross-core sync.
"""

import os
import sys

import numpy as np

for _p in (
    "/root/.axon_site",
    "/root/.axon_site/_ro/trn_rl_repo",
    "/root/.axon_site/_ro/pypackages",
    "/opt/trn_rl_repo",
):
    if os.path.isdir(_p) and _p not in sys.path:
        sys.path.append(_p)

from contextlib import ExitStack

import concourse.bacc as bacc
import concourse.mybir as mybir
from concourse import tile
from concourse.bass_utils import run_bass_kernel_spmd

N_CORES = 8
N_ROWS = 16384
N_COLS = 512
SHARD_ROWS = N_ROWS // N_CORES  # 2048
P = 128
N_CHUNKS = 8
CHUNK_ROWS = SHARD_ROWS // N_CHUNKS  # 256
T_PER_CHUNK = CHUNK_ROWS // P  # 2

_nc_cache = None

# Set by kernel() after each run; test harnesses can read exec_time_ns etc.
LAST_RESULTS = None


def _build():
    nc = bacc.Bacc("TRN2", target_bir_lowering=False, debug=False)
    a = nc.dram_tensor("a", [SHARD_ROWS, N_COLS], mybir.dt.float32, kind="ExternalInput")
    out_cs = nc.dram_tensor("out_cs", [P, N_COLS], mybir.dt.float32, kind="ExternalOutput")
    out_sq = nc.dram_tensor("out_sq", [P, N_CHUNKS], mybir.dt.float32, kind="ExternalOutput")

    with tile.TileContext(nc) as tc, ExitStack() as ctx:
        io_pool = ctx.enter_context(tc.tile_pool(name="io", bufs=N_CHUNKS))
        work_pool = ctx.enter_context(tc.tile_pool(name="work", bufs=3))
        acc_pool = ctx.enter_context(tc.tile_pool(name="accp", bufs=2))
        scr_pool = ctx.enter_context(tc.tile_pool(name="scrp", bufs=2))
        stat_pool = ctx.enter_context(tc.tile_pool(name="stat", bufs=1))

        stats = stat_pool.tile([P, N_CHUNKS], mybir.dt.float32)

        acc = None
        for c in range(N_CHUNKS):
            chunk = io_pool.tile([P, T_PER_CHUNK, N_COLS], mybir.dt.float32, tag="chunk")
            src = a[c * CHUNK_ROWS : (c + 1) * CHUNK_ROWS, :].rearrange(
                "(t p) d -> p t d", p=P
            )
            nc.sync.dma_start(out=chunk[:], in_=src)

            # Sum of squares of the whole chunk on ACT: scratch gets the squared
            # values (discarded), accum_out the per-partition reduction.
            scr = scr_pool.tile([P, T_PER_CHUNK * N_COLS], mybir.dt.float32, tag="scr")
            flat = chunk.rearrange("p t d -> p (t d)")
            nc.scalar.activation(
                scr[:],
                flat,
                mybir.ActivationFunctionType.Square,
                accum_out=stats[:, c : c + 1],
            )

            # Column-sum partial on DVE: pair-add the two row-tiles of the
            # chunk, then fold into the running accumulator (ping-pong bufs).
            if acc is None:
                acc = acc_pool.tile([P, N_COLS], mybir.dt.float32, tag="acc")
                nc.vector.tensor_add(acc[:], chunk[:, 0, :], chunk[:, 1, :])
            else:
                pair = work_pool.tile([P, N_COLS], mybir.dt.float32, tag="pair")
                nc.vector.tensor_add(pair[:], chunk[:, 0, :], chunk[:, 1, :])
                nxt = acc_pool.tile([P, N_COLS], mybir.dt.float32, tag="acc")
                nc.vector.tensor_add(nxt[:], acc[:], pair[:])
                acc = nxt

        nc.sync.dma_start(out=out_cs.ap(), in_=acc[:])
        nc.sync.dma_start(out=out_sq.ap(), in_=stats[:])

    nc.compile()
    return nc


def _get_nc():
    global _nc_cache
    if _nc_cache is None:
        _nc_cache = _build()
    return _nc_cache


def kernel(A: np.ndarray) -> np.ndarray:
    global LAST_RESULTS
    a = np.ascontiguousarray(np.asarray(A, dtype=np.float32))
    assert a.shape == (N_ROWS, N_COLS), a.shape

    nc = _get_nc()
    shards = a.reshape(N_CORES, SHARD_ROWS, N_COLS)
    in_maps = [{"a": np.ascontiguousarray(shards[c])} for c in range(N_CORES)]
    results = run_bass_kernel_spmd(nc, in_maps, list(range(N_CORES)))
    LAST_RESULTS = results

    cs = np.zeros(N_COLS, dtype=np.float64)
    sq = 0.0
    for r in results.results:
        cs += r["out_cs"].astype(np.float64).sum(axis=0)
        sq += float(r["out_sq"].astype(np.float64).sum())
    total = float(cs @ cs)
    denom = float(N_ROWS) * float(N_ROWS - 1)
    return np.asarray((total - sq) / denom, dtype=np.float32)
